# revision 2
# baseline (speedup 1.0000x reference)
"""Trainium2 Bass kernel for nn_ConvGraphSelfLoop.

out = where(any(adj>=0, axes -1,-2), relu(features @ W + b), features)

Sharding: B*V = 65536 vertices split evenly across 8 NeuronCores (8192
each); W/bias replicated; no cross-core communication.

Per core (fully unrolled, 64 token-tiles of 128 vertices):
  - PE transposes x [128,1024] -> xT chunks (fp32, exact) into PSUM
  - ACT evicts PSUM -> SBUF rounding to float32r (1 cyc/row matmul rate)
  - PE: per 512-wide u-half: K=1 bias matmul + 8 accumulated f32r matmuls
  - ACT: relu(psum * mask) eviction (mask zeroes invalid rows)
  - DVE: mask from adjacency; xc = x*(1-mask); out = relu_part + xc
x is loaded twice (separate tiles for the PE and DVE consumers) to keep
DMA WAR chains simple. Built on bacc.Bacc: compile() legalizes the
1-wait-per-instruction TRN2 constraint (split_sync_waits).
"""
import numpy as np
import concourse.bass as bass
import concourse.bacc as bacc
import concourse.mybir as mybir
import concourse.tile as tile
from concourse.bass_utils import run_bass_kernel_spmd

B, V, E, NN = 4, 16384, 4, 32
F, U = 1024, 1024
NCORES = 8
T = B * V // NCORES          # 8192 tokens per core
P = 128
NT = T // P                  # 64 token tiles
C = F // P                   # 8 contraction chunks
NH = U // 512                # 2 u-halves
BUFS = 3

f32 = mybir.dt.float32
f32r = mybir.dt.float32r
i32 = mybir.dt.int32
AF = mybir.ActivationFunctionType
ALU = mybir.AluOpType


def _build():
    nc = bacc.Bacc("TRN2", target_bir_lowering=False, debug=False,
                   num_devices=NCORES)
    feat_d = nc.dram_tensor("features", [T, F], f32, kind="ExternalInput")
    adj_d = nc.dram_tensor("adjacency", [T, E * NN], i32, kind="ExternalInput")
    w_d = nc.dram_tensor("weight", [F, U], f32, kind="ExternalInput")
    bias_d = nc.dram_tensor("bias", [1, U], f32, kind="ExternalInput")
    out_d = nc.dram_tensor("out", [T, U], f32, kind="ExternalOutput")

    with tile.TileContext(nc) as tc:
        with tc.tile_pool(name="const", bufs=1) as const, \
             tc.tile_pool(name="xp", bufs=BUFS) as xp, \
             tc.tile_pool(name="xd", bufs=BUFS) as xd, \
             tc.tile_pool(name="xtp", bufs=BUFS) as xtp, \
             tc.tile_pool(name="op", bufs=BUFS) as op, \
             tc.tile_pool(name="ap", bufs=BUFS) as apool, \
             tc.tile_pool(name="mp", bufs=BUFS) as mp, \
             tc.tile_pool(name="psT", bufs=2, space="PSUM") as psT, \
             tc.tile_pool(name="psO", bufs=2, space="PSUM") as psO:

            # ---- startup constants ----
            w_st = const.tile([P, C * U], f32)
            for c in range(C):
                nc.sync.dma_start(w_st[:, c * U:(c + 1) * U],
                                  w_d.ap()[c * P:(c + 1) * P, :])
            w_r = const.tile([P, C * U], f32r)
            for c in range(C):
                nc.scalar.copy(w_r[:, c * U:(c + 1) * U],
                               w_st[:, c * U:(c + 1) * U])
            bias_st = const.tile([1, U], f32)
            nc.sync.dma_start(bias_st[:], bias_d.ap())
            bias_r = const.tile([1, U], f32r)
            nc.scalar.copy(bias_r[:], bias_st[:])
            ones_st = const.tile([1, P], f32)
            nc.gpsimd.memset(ones_st[:], 1.0)
            ones_r = const.tile([1, P], f32r)
            nc.scalar.copy(ones_r[:], ones_st[:])
            ident = const.tile([P, P], f32)
            nc.gpsimd.memset(ident[:], 0.0)
            nc.gpsimd.affine_select(
                out=ident[:], in_=ident[:],
                compare_op=ALU.not_equal, fill=1.0, base=0,
                pattern=[[-1, P]], channel_multiplier=1,
            )

            for t in range(NT):
                rows = slice(t * P, (t + 1) * P)
                # ---- DMA loads ----
                x_pe = xp.tile([P, F], f32, tag="x_pe")
                nc.sync.dma_start(x_pe[:], feat_d.ap()[rows, :])
                x_dve = xd.tile([P, F], f32, tag="x_dve")
                nc.sync.dma_start(x_dve[:], feat_d.ap()[rows, :])
                adj_t = apool.tile([P, E * NN], i32, tag="adj")
                nc.sync.dma_start(adj_t[:], adj_d.ap()[rows, :])

                # ---- PE: bias matmuls open the psum accumulation ----
                po = psO.tile([P, U], f32, tag="po")
                for h in range(NH):
                    nc.tensor.matmul(po[:, h * 512:(h + 1) * 512],
                                     ones_r[:], bias_r[:, h * 512:(h + 1) * 512],
                                     start=True, stop=False)

                # ---- PE: transposes (fp32 exact) ----
                pT = psT.tile([P, 2 * 512], f32, tag="pT")
                for c in range(C):
                    nc.tensor.transpose(pT[:, c * P:(c + 1) * P],
                                        x_pe[:, c * P:(c + 1) * P], ident[:])

                # ---- ACT: evict transposes to SBUF as f32r (one big copy) ----
                xT_r = xtp.tile([P, F], f32r, tag="xT_r")
                nc.scalar.copy(xT_r[:], pT[:])

                # ---- DVE: mask pipeline ----
                mx = mp.tile([P, 1], i32, tag="mx")
                nc.vector.tensor_reduce(mx[:], adj_t[:],
                                        axis=mybir.AxisListType.X, op=ALU.max)
                m_f = mp.tile([P, 1], f32, tag="m_f")
                nc.vector.tensor_scalar(m_f[:], mx[:], 0, None, ALU.is_ge)
                minv = mp.tile([P, 1], f32, tag="minv")
                nc.vector.tensor_scalar(minv[:], m_f[:], -1.0, 1.0,
                                        ALU.mult, ALU.add)
                xc = xd.tile([P, F], f32, tag="xc")
                nc.vector.tensor_scalar(xc[:], x_dve[:], minv[:], None, ALU.mult)

                # ---- ACT: mask copy (washes DVE dep into ACT stream) ----
                m_act = mp.tile([P, 1], f32, tag="m_act")
                nc.scalar.copy(m_act[:], m_f[:])

                # ---- PE: main f32r matmuls ----
                for h in range(NH):
                    for c in range(C):
                        nc.tensor.matmul(
                            po[:, h * 512:(h + 1) * 512],
                            xT_r[:, c * P:(c + 1) * P],
                            w_r[:, c * U + h * 512: c * U + (h + 1) * 512],
                            start=False, stop=(c == C - 1))

                # ---- ACT: relu(psum * mask) -> r_t ----
                r_t = op.tile([P, U], f32, tag="r_t")
                nc.scalar.activation(r_t[:], po[:], AF.Relu, scale=m_act[:])

                # ---- DVE: out = r_t + xc ----
                out_t = op.tile([P, U], f32, tag="out_t")
                nc.vector.tensor_tensor(out=out_t[:], in0=r_t[:], in1=xc[:],
                                        op=ALU.add)

                # ---- DMA store ----
                nc.sync.dma_start(out_d.ap()[rows, :], out_t[:])

    nc.compile()
    return nc


_nc_cache = None


def _get_nc():
    global _nc_cache
    if _nc_cache is None:
        _nc_cache = _build()
    return _nc_cache


def _make_in_maps(inputs):
    feats = np.ascontiguousarray(
        inputs["features"].reshape(B * V, F), dtype=np.float32)
    adj = np.ascontiguousarray(
        inputs["adjacency"].reshape(B * V, E * NN), dtype=np.int32)
    w = np.ascontiguousarray(inputs["kernel"], dtype=np.float32)
    b = np.ascontiguousarray(inputs["bias"].reshape(1, U), dtype=np.float32)
    in_maps = []
    for i in range(NCORES):
        s = slice(i * T, (i + 1) * T)
        in_maps.append({
            "features": feats[s],
            "adjacency": adj[s],
            "weight": w,
            "bias": b,
        })
    return in_maps


def kernel(adjacency, features, kernel, bias):
    nc = _get_nc()
    in_maps = _make_in_maps({"adjacency": adjacency, "features": features,
                             "kernel": kernel, "bias": bias})
    res = run_bass_kernel_spmd(nc, in_maps, list(range(NCORES)))
    out = np.concatenate([res.results[i]["out"] for i in range(NCORES)], axis=0)
    return out.reshape(B, V, U).astype(features.dtype)


if __name__ == "__main__":
    import io, contextlib, re
    nc = _build()
    buf = io.StringIO()
    with contextlib.redirect_stdout(buf):
        nc.print_concise(deps=True)
    bad = 0
    for line in buf.getvalue().splitlines():
        n = len(re.findall(r"wait:S\[", line))
        if n > 1:
            bad += 1
            if bad <= 8:
                print("MULTI-WAIT:", line[:200])
    print(f"instructions with >1 wait: {bad}")



# revision 3
# speedup vs baseline: 1.9961x; 1.9961x over previous
"""Trainium2 Bass kernel for nn_ConvGraphSelfLoop.

out = where(any(adj>=0, axes -1,-2), relu(features @ W + b), features)

Strategy (device does the GEMM, host does layout + select):
  - A vertex is "valid" iff any adjacency entry >= 0. Invalid vertices
    pass their input features through untouched — the host writes those
    directly from the fp32 input, so the device only transforms valid
    vertices (~75% of 65536).
  - Host compacts the valid vertices, casts to fp16, transposes to
    xT [F, n] and splits them evenly across 8 cores (capacity 6656
    tokens/core = 13 blocks of 512; valid count 49152 +- 111, so 6656
    per core is a +37 sigma bound. Any overflow beyond capacity is
    computed on the host — correctness never depends on the bound).
  - Device computes outT = relu(W^T @ xT + b) in transposed space:
      * W [F, U] already has the contraction dim on partitions, so W
        chunks are the stationary operand — NO PE transposes at all.
      * bias lands on partitions (u-chunks), so it rides the ACT
        eviction (activation bias operand) — no bias matmuls.
      * fp16 operands: 1 cyc/row PE rate (same as bf16), half the DMA.
  - Per core: 13 token-blocks of 512, grouped in superblocks of 2048
    so each stationary W chunk serves 4 consecutive matmuls.
    PE work = 832 matmuls x 512 rows ~= 178 us; ACT evicts psum with
    relu+bias; DMA in/out ~28 MB fully overlapped.
"""
import numpy as np
import concourse.bass as bass
import concourse.bacc as bacc
import concourse.mybir as mybir
import concourse.tile as tile
from concourse.bass_utils import run_bass_kernel_spmd

B, V, E, NN = 4, 16384, 4, 32
F, U = 1024, 1024
NCORES = 8
P = 128
BLK = 512                    # tokens per psum bank / matmul free dim
NBLK = 13                    # token-blocks per core (capacity 6656)
CAP = NBLK * BLK             # 6656 tokens per core
SUPERS = [(0, 2048), (2048, 2048), (4096, 2048), (6144, 512)]
CF = F // P                  # 8 contraction chunks
CU = U // P                  # 8 output-partition chunks

f32 = mybir.dt.float32
f16 = mybir.dt.float16
AF = mybir.ActivationFunctionType


def _build():
    nc = bacc.Bacc("TRN2", target_bir_lowering=False, debug=False,
                   num_devices=NCORES)
    xt_d = nc.dram_tensor("xt", [F, CAP], f16, kind="ExternalInput")
    w_d = nc.dram_tensor("weight", [F, U], f16, kind="ExternalInput")
    bias_d = nc.dram_tensor("bias", [P, CU], f32, kind="ExternalInput")
    out_d = nc.dram_tensor("outT", [U, CAP], f16, kind="ExternalOutput")

    with tile.TileContext(nc) as tc:
        with tc.tile_pool(name="const", bufs=1) as const, \
             tc.tile_pool(name="xp", bufs=2) as xp, \
             tc.tile_pool(name="op", bufs=3) as op, \
             tc.tile_pool(name="psp", bufs=2, space="PSUM") as psp:

            # ---- resident constants: W chunks + bias ----
            # w_sb[:, f*U + j] = W[f*P + p, j]  (slab f = W rows f*P..)
            w_sb = const.tile([P, CF * U], f16)
            for f in range(CF):
                nc.sync.dma_start(w_sb[:, f * U:(f + 1) * U],
                                  w_d.ap()[f * P:(f + 1) * P, :])
            bias_sb = const.tile([P, CU], f32)
            nc.sync.dma_start(bias_sb[:], bias_d.ap())

            for off, W in SUPERS:
                nb = W // BLK
                # xs holds CF f-chunk slabs side by side: chunk f at
                # cols [f*W, (f+1)*W)
                xs = xp.tile([P, CF * W], f16, tag="xs")
                for f in range(CF):
                    nc.sync.dma_start(xs[:, f * W:(f + 1) * W],
                                      xt_d.ap()[f * P:(f + 1) * P,
                                                off:off + W])
                for u in range(CU):
                    ps = psp.tile([P, W], f32, tag="ps")
                    for f in range(CF):
                        lhsT = w_sb[:, f * U + u * P: f * U + (u + 1) * P]
                        for b in range(nb):
                            nc.tensor.matmul(
                                ps[:, b * BLK:(b + 1) * BLK],
                                lhsT,
                                xs[:, f * W + b * BLK: f * W + (b + 1) * BLK],
                                start=(f == 0), stop=(f == CF - 1))
                    r = op.tile([P, W], f16, tag="r")
                    nc.scalar.activation(r[:], ps[:], AF.Relu,
                                         bias=bias_sb[:, u:u + 1])
                    nc.sync.dma_start(out_d.ap()[u * P:(u + 1) * P,
                                                 off:off + W], r[:])

    nc.compile()
    return nc


_nc_cache = None


def _get_nc():
    global _nc_cache
    if _nc_cache is None:
        _nc_cache = _build()
    return _nc_cache


def _preprocess(inputs):
    """Host-side: mask, compaction, fp16 transpose, per-core split."""
    feats2 = np.asarray(inputs["features"], dtype=np.float32).reshape(B * V, F)
    adj2 = np.asarray(inputs["adjacency"]).reshape(B * V, E * NN)
    valid = adj2.max(axis=1) >= 0
    idx = np.flatnonzero(valid)
    dev_idx = idx[:NCORES * CAP]          # device-computed valid tokens
    ovf_idx = idx[NCORES * CAP:]          # host fallback (statistically never)

    w16 = np.ascontiguousarray(inputs["kernel"], dtype=np.float16)
    bias = np.asarray(inputs["bias"], dtype=np.float32).reshape(-1)
    bias_dev = np.ascontiguousarray(bias.reshape(CU, P).T, dtype=np.float32)

    n = dev_idx.size
    counts = [(n + NCORES - 1 - i) // NCORES for i in range(NCORES)]
    starts = np.cumsum([0] + counts)
    in_maps, core_idx = [], []
    for i in range(NCORES):
        ci = dev_idx[starts[i]:starts[i + 1]]
        core_idx.append(ci)
        xti = np.zeros((F, CAP), dtype=np.float16)
        if ci.size:
            xti[:, :ci.size] = feats2[ci].T.astype(np.float16)
        in_maps.append({"xt": xti, "weight": w16, "bias": bias_dev})
    return feats2, valid, core_idx, ovf_idx, in_maps


def _make_in_maps(inputs):
    return _preprocess(inputs)[4]


def kernel(adjacency, features, kernel, bias):
    nc = _get_nc()
    inputs = {"adjacency": adjacency, "features": features,
              "kernel": kernel, "bias": bias}
    feats2, valid, core_idx, ovf_idx, in_maps = _preprocess(inputs)
    res = run_bass_kernel_spmd(nc, in_maps, list(range(NCORES)))

    out = np.empty((B * V, U), dtype=np.float32)
    out[~valid] = feats2[~valid]
    for i in range(NCORES):
        ci = core_idx[i]
        if ci.size:
            oT = res.results[i]["outT"]
            out[ci] = oT[:, :ci.size].T.astype(np.float32)
    if ovf_idx.size:
        w32 = np.asarray(kernel, dtype=np.float32)
        b32 = np.asarray(bias, dtype=np.float32).reshape(-1)
        out[ovf_idx] = np.maximum(feats2[ovf_idx] @ w32 + b32, 0.0)
    return out.reshape(B, V, U)
